# revision 51
# baseline (speedup 1.0000x reference)
"""BiLSTM-CRF loss kernel for 8 Trainium2 NeuronCores.

Math (per sequence):
  NLL = log Z - gold
  log Z:  forward algorithm over L=1024 steps, T=32 tags.
  gold:   score of the labelled path (gathered host-side, summed on device).

Device formulation (linear domain):
  a_{l+1} = diag(exp(f_l - mu)) @ E^T @ a_l      E[i,j] = exp(trans[i,j])

The L=1024 chain is split into S=64 independent segments of 16 steps,
which cuts the sequential depth from 512 (baseline fwd+bwd halves) to
TSS=17 supersteps.  Positive CRF chains contract in the Hilbert
projective metric (worst case ~tanh(max|trans|) < 0.4 per step;
empirically far faster through diag(exp(feat))), so every segment except
the first starts BURN=1 step early from a uniform vector and has
forgotten the wrong start by its junction (validated host-side: adds
< 1e-5 to a 5e-5 total relative error at 2e-2 tolerance).  Junction and
end log-sums are captured per segment; telescoping their differences
gives log Z exactly:
  logZ = sum_s [ln sum(v_end^s) - ln sum(v_junction^s)] + mu * L
Segment 0 starts exactly from one-hot(START) (row START of E is zero so
that vector cannot be produced mid-chain); it needs no burn step, its
end capture is at t=16, and it idles on fx=1 columns afterwards.  The
STOP transition is folded into the last staged column of segment 63.

Layout per core (128 sequences):
  Supersteps t = 0..16 advance all 64 segments one step.  Partitions pack
  4 segment-slots x 32 tags; columns pack 16 quads x 128 seqs = 2048 cols,
  split into 2 pairs of PSUM banks (1024 cols each).  Per superstep and
  pair: two bf16 matmuls [128,128]@[128,512] (stationary block-diag E^T,
  shared by every step so weights are loaded once) into adjacent banks,
  then ONE wide DVE multiply [128,1024] with the staged exp feats (fp32
  PSUM can only be read by DVE on TRN2 -- GpSimd cannot access PSUM --
  so wide muls amortize the DVE per-instruction PSUM-access cost).  The
  DVE multiplies are the bottleneck (~95% busy); the staged-feat DMA and
  the ACT exp stream are pipelined underneath via the chunk schedule.
  Per-segment sums are captured with ones-matmuls + Ln, and the final
  per-sequence combine transposes (el - jl) via 32 tiny accumulating
  K=4 matmuls.

Host-side staging only reorders/masks/gathers the inputs: feats are laid
out as [(slot,tag), superstep, col] bf16 shifted by -mu; exp() happens on
device.  The gold increments (feats[l,tag_l] + trans[tag_l,tag_{l-1}])
are host-gathered and summed on device.

Baseline: 325 us.  This kernel: ~53.1 us (TimelineSim; rel err 5.3e-5).
"""

import sys

sys.path.insert(0, "/opt/trn_rl_repo")

import numpy as np
import ml_dtypes

B, L, T = 1024, 1024, 32
START, STOP = 30, 31
NCORES = 8
BS = B // NCORES          # sequences per core
SEG_LEN = 16
S = L // SEG_LEN          # segments per sequence
BURN = 1
TSS = SEG_LEN + BURN      # supersteps
NQ = S // 4               # quads (column blocks of 128 seqs)
COLS = NQ * BS            # 2048 columns per superstep
NG = 4                    # matmul groups
GCOLS = COLS // NG        # 512 cols per group = one PSUM bank fp32
MU = 3.9
MU_CONST = MU * L
NPAIR = 2                 # psum-bank pairs; each pair = 1024 cols, 1 DVE mul
PCOLS = COLS // NPAIR
GOLD_W = 1028             # 1024 emit+trans, 1 stop, 3 pad
# chunk schedule (supersteps per staged DMA chunk)
CHUNK_SCHED = [1, 1, 1, 1, 2, 2, 2, 2, 2, 1, 1, 1]
assert sum(CHUNK_SCHED) == TSS
DMA_AHEAD = 5   # issue chunk DMA when its first superstep is this close
EXP_AHEAD = 3   # issue chunk exp likewise
# head DMA issue order: ("c", k)=chunk k, ("i", p)=pair-p init
EXP_SPLIT_K = 1   # chunks up to this index get pair-split exp instructions
EL_SPLIT = True   # per-group end captures with pipelined Lns

_compiled = None


def _patch_act_tables(mybir):
    """Make the act-table selector pick the one set containing BOTH Exp and
    Ln (natural_log_exp_and_others) so the kernel needs a single table load
    instead of swapping Exp<->Ln tables (1283 ns each) mid-stream.  Only the
    selector's view is filtered; emitted act_func_set_ids still index the real
    act_info.json.  Returns an undo callback."""
    import concourse.bacc as bacc_mod

    orig = bacc_mod.get_activation_tables
    keep = "natural_log_exp_and_others"
    exp_ln = {mybir.ActivationFunctionType.Exp, mybir.ActivationFunctionType.Ln}

    def patched(arch):
        tabs = orig(arch)
        return {
            name: (s if name == keep else set(s) - exp_ln)
            for name, s in tabs.items()
        }

    bacc_mod.get_activation_tables = patched

    def undo():
        bacc_mod.get_activation_tables = orig

    return undo


def _build_nc():
    import concourse.bacc as bacc
    import concourse.tile as tile
    import concourse.mybir as mybir
    from concourse.bass import AP

    fp32 = mybir.dt.float32
    bf16 = mybir.dt.bfloat16
    Exp = mybir.ActivationFunctionType.Exp
    Ln = mybir.ActivationFunctionType.Ln

    nc = bacc.Bacc(
        "TRN2",
        target_bir_lowering=False,
        debug=False,
        enable_asserts=False,
        num_devices=NCORES,
    )
    staged_d = nc.dram_tensor("staged", [128, TSS * COLS], bf16, kind="ExternalInput").ap()
    gold_d = nc.dram_tensor("gold", [BS, GOLD_W], fp32, kind="ExternalInput").ap()
    trans_d = nc.dram_tensor("trans", [T, T], fp32, kind="ExternalInput").ap()
    idn_d = nc.dram_tensor("idn", [4, 10], fp32, kind="ExternalInput").ap()
    out_d = nc.dram_tensor("out", [BS, 1], fp32, kind="ExternalOutput").ap()

    from contextlib import ExitStack

    with tile.TileContext(nc) as tc, ExitStack() as ctx:
        singles = ctx.enter_context(tc.tile_pool(name="singles", bufs=1))
        st_pool = ctx.enter_context(tc.tile_pool(name="staged", bufs=6))
        fx_pool = ctx.enter_context(tc.tile_pool(name="fexp", bufs=4))
        rhs_pools = [
            ctx.enter_context(tc.tile_pool(name=f"rhs{p}", bufs=2))
            for p in range(NPAIR)
        ]
        ps_pools = [
            ctx.enter_context(tc.tile_pool(name=f"ps{p}", bufs=1, space="PSUM"))
            for p in range(NPAIR)
        ]
        cap_pool = ctx.enter_context(tc.tile_pool(name="cap", bufs=1, space="PSUM"))
        sm_pool = ctx.enter_context(tc.tile_pool(name="small", bufs=2))

        # ---- head DMAs (sync queue): trans (tiny, gates w1) first --------
        trans_rep = singles.tile([128, T], fp32, tag="trans_rep")
        rep_ap = AP(
            tensor=trans_d.tensor, offset=trans_d.offset,
            ap=[[0, 4]] + [list(x) for x in trans_d.ap],
        )
        nc.gpsimd.dma_start(out=trans_rep[:], in_=rep_ap)

        # E_rep[32k+i, j] = exp(trans[i, j]); e_rept[32k+j, i] = exp(trans[i, j])
        e_rep = singles.tile([128, T], bf16, tag="e_rep")
        nc.scalar.activation(e_rep[:], trans_rep[:], Exp)
        e_rept = singles.tile([128, T], bf16, tag="e_rept")
        nc.vector.transpose(e_rept[:], e_rep[:])

        # W1: block-diag stationary, all four slots forward-form
        w1 = singles.tile([128, 128], bf16, tag="w1")
        nc.vector.memset(w1[:], 0.0)
        for k in range(4):
            nc.vector.tensor_copy(
                w1[32 * k : 32 * (k + 1), 32 * k : 32 * (k + 1)],
                e_rept[32 * k : 32 * (k + 1), :],
            )

        # ones4: lhsT for per-slot column sums  [128, 4]
        ones4 = singles.tile([128, 4], bf16, tag="ones4")
        nc.vector.memset(ones4[:], 0.0)
        for k in range(4):
            nc.vector.memset(ones4[32 * k : 32 * (k + 1), k : k + 1], 1.0)
        # [+I | -I | +1s | -1s] fp32 (DMA deferred to a DMA lull below)
        idn = singles.tile([4, 10], fp32, tag="idn")

        # log-sum capture tiles (fp32, partitions 0:4)
        jl = singles.tile([4, COLS], fp32, tag="jl")
        el = singles.tile([4, COLS], fp32, tag="el")
        el0 = singles.tile([4, 128], fp32, tag="el0")

        # ---- chunk loading: DMA prefetched 2 chunks ahead, exp 1 ahead ----
        chunk_bounds = []
        c0 = 0
        for clen in CHUNK_SCHED:
            chunk_bounds.append((c0, clen))
            c0 += clen
        st_tiles = {}
        fx_tiles = {}

        def issue_dma(k):
            c0, clen = chunk_bounds[k]
            st = st_pool.tile([128, clen * COLS], bf16, tag="st", name=f"st_{c0}")
            nc.sync.dma_start(
                out=st[:], in_=staged_d[:, c0 * COLS : (c0 + clen) * COLS]
            )
            st_tiles[k] = st

        def issue_exp(k):
            c0, clen = chunk_bounds[k]
            st = st_tiles[k]
            fx = fx_pool.tile([128, clen * COLS], bf16, tag="fx", name=f"fx_{c0}")
            if k <= EXP_SPLIT_K:
                # split pair-aligned so the first muls wait on half only
                nc.scalar.activation(fx[:, 0:PCOLS], st[:, 0:PCOLS], Exp)
                nc.scalar.activation(fx[:, PCOLS:], st[:, PCOLS:], Exp)
            else:
                nc.scalar.activation(fx[:], st[:], Exp)
            fx_tiles[k] = fx

        def capture(dst, p, nm):
            """Sum each 32-row slot of pair p's state into dst's pair slice
            (fp32 sbuf) via ones-matmuls into a bank-aligned psum tile + Ln."""
            cap = cap_pool.tile([4, PCOLS], fp32, tag=f"cap{p}", name=nm)
            for h in range(PCOLS // GCOLS):
                nc.tensor.matmul(
                    cap[:, h * GCOLS : (h + 1) * GCOLS], ones4[:],
                    rhs[p][:, h * GCOLS : (h + 1) * GCOLS],
                    start=True, stop=True,
                )
            nc.scalar.activation(dst[0:4, p * PCOLS : (p + 1) * PCOLS], cap[:], Ln)

        # ---- head: no init DMA at all -- superstep 0 is computed as
        # fx0 * (E^T @ init) where E^T @ ones = column-sums of w1 (one tiny
        # matmul) and E^T @ onehot(START) = column START of e_rep ----------
        issue_dma(0)
        issue_dma(1)
        issue_exp(0)
        onescol = singles.tile([128, 1], bf16, tag="onescol")
        nc.vector.memset(onescol[:], 1.0)
        c128p = ps_pools[0].tile([128, 1], fp32, tag="ps0", name="c128p")
        nc.tensor.matmul(c128p[:], w1[:], onescol[:], start=True, stop=True)
        c128 = singles.tile([128, 1], fp32, tag="c128")
        nc.vector.tensor_copy(c128[:], c128p[:])
        estart = singles.tile([32, 1], fp32, tag="estart")
        nc.vector.tensor_copy(estart[:], e_rep[0:32, START : START + 1])
        rhs = [None, None]

        # ---- main loop ----------------------------------------------------
        next_dma = 2
        next_exp = 1
        chunk_idx = 0
        jl_caps = []

        for t in range(TSS):
            while next_dma < len(chunk_bounds) and chunk_bounds[next_dma][0] <= t + DMA_AHEAD:
                issue_dma(next_dma)
                next_dma += 1
            while next_exp < len(chunk_bounds) and chunk_bounds[next_exp][0] <= t + EXP_AHEAD:
                issue_exp(next_exp)
                next_exp += 1
            if t == 2:
                # sync queue => strictly after the early chunk DMAs in the
                # DMA-engine FIFO; lands in the catch-up lull ~10-15us
                gold = singles.tile([BS, GOLD_W], fp32, tag="gold")
                nc.sync.dma_start(out=gold[:], in_=gold_d)
                nc.sync.dma_start(out=idn[:], in_=idn_d)
            if t >= chunk_bounds[chunk_idx][0] + chunk_bounds[chunk_idx][1]:
                chunk_idx += 1
                del st_tiles[chunk_idx - 1], fx_tiles[chunk_idx - 1]
            fx = fx_tiles[chunk_idx]
            lt = t - chunk_bounds[chunk_idx][0]

            if t == 12:
                for p in range(NPAIR):
                    nc.scalar.activation(
                        jl[0:4, p * PCOLS : (p + 1) * PCOLS], jl_caps[p][:], Ln
                    )
                nc.vector.memset(jl[0:1, 0:128], 0.0)  # seg 0: exact start
            if t == SEG_LEN:
                capg0 = cap_pool.tile([4, PCOLS], fp32, tag="cap0", name="el0_cap")
                nc.tensor.matmul(capg0[:, 0:128], ones4[:], rhs[0][:, 0:128],
                                 start=True, stop=True)
                nc.scalar.activation(el0[:], capg0[:, 0:128], Ln)

            for p in range(NPAIR):
                nr = rhs_pools[p].tile([128, PCOLS], bf16, tag=f"rhs{p}", name=f"rhs{p}_{t}")
                base = lt * COLS + p * PCOLS
                if t == 0:
                    nc.vector.tensor_scalar_mul(
                        nr[:], fx[:, base : base + PCOLS], c128[:]
                    )
                    if p == 0:
                        # segment 0 starts from one-hot(START), not ones
                        nc.vector.tensor_scalar_mul(
                            nr[0:32, 0:128], fx[0:32, base : base + 128],
                            estart[:],
                        )
                else:
                    ps = ps_pools[p].tile([128, PCOLS], fp32, tag=f"ps{p}", name=f"ps{p}_{t}")
                    for h in range(PCOLS // GCOLS):
                        nc.tensor.matmul(
                            ps[:, h * GCOLS : (h + 1) * GCOLS], w1[:],
                            rhs[p][:, h * GCOLS : (h + 1) * GCOLS],
                            start=True, stop=True,
                        )
                    nc.vector.tensor_mul(nr[:], ps[:], fx[:, base : base + PCOLS])
                if t == BURN:
                    # junction capture of the PRE-step state (rhs, not nr):
                    # emitted after this superstep's matmuls so the ramp-
                    # critical PE queue isn't delayed (read-read on rhs).
                    cap = cap_pool.tile([4, PCOLS], fp32, tag=f"cap{p}", name=f"jl_cap{p}")
                    for h in range(PCOLS // GCOLS):
                        nc.tensor.matmul(
                            cap[:, h * GCOLS : (h + 1) * GCOLS], ones4[:],
                            rhs[p][:, h * GCOLS : (h + 1) * GCOLS],
                            start=True, stop=True,
                        )
                    jl_caps.append(cap)
                rhs[p] = nr
                if t == TSS - 1:
                    if EL_SPLIT:
                        # per-group captures: each 512-col Ln starts right
                        # after its own ones-matmul, pipelining the tail
                        cap = cap_pool.tile([4, PCOLS], fp32, tag=f"cap{p}",
                                            name=f"el_cap{p}")
                        for h in range(PCOLS // GCOLS):
                            cs = slice(h * GCOLS, (h + 1) * GCOLS)
                            nc.tensor.matmul(cap[:, cs], ones4[:], rhs[p][:, cs],
                                             start=True, stop=True)
                            nc.scalar.activation(
                                el[0:4, p * PCOLS + h * GCOLS :
                                   p * PCOLS + (h + 1) * GCOLS],
                                cap[:, cs], Ln)
                    else:
                        capture(el, p, f"el_cap{p}")

        # ---- final combine ------------------------------------------------
        nc.vector.tensor_copy(el[0:1, 0:128], el0[0:1, 0:128])  # seg 0 end @t=16

        # gold reduced and pre-folded with the mu constant during DVE idle
        gred = sm_pool.tile([128, 1], fp32, tag="gred")
        nc.vector.tensor_reduce(
            gred[:], gold[:], axis=mybir.AxisListType.X, op=mybir.AluOpType.add
        )
        gmc = sm_pool.tile([128, 1], fp32, tag="gmc")
        nc.vector.tensor_scalar(
            gmc[:], gred[:], -1.0, float(MU_CONST),
            op0=mybir.AluOpType.mult, op1=mybir.AluOpType.add,
        )

        # zacc[seq] = sum over slots and quads of (el - jl): the transposing
        # K=4 matmuls contract slots directly via +/-ones rhs vectors
        zacc = ps_pools[0].tile([128, 1], fp32, tag="ps0", name="zacc")
        n_mm = 2 * NQ
        i = 0
        for q in range(NQ):
            cs = slice(128 * q, 128 * (q + 1))
            nc.tensor.matmul(zacc[:], el[:, cs], idn[:, 8:9],
                             start=(i == 0), stop=(i == n_mm - 1)); i += 1
            nc.tensor.matmul(zacc[:], jl[:, cs], idn[:, 9:10],
                             start=(i == 0), stop=(i == n_mm - 1)); i += 1

        res = sm_pool.tile([128, 1], fp32, tag="res")
        nc.vector.tensor_add(res[:], zacc[:], gmc[:])
        nc.sync.dma_start(out=out_d[:], in_=res[:])

    undo = _patch_act_tables(mybir)
    try:
        nc.compile()
    finally:
        undo()
    return nc


def _stage_core(feats_c, tags_c, trans):
    """feats_c [BS,L,T] f32, tags_c [BS,L] -> staged [128, TSS*COLS] bf16,
    init [128, COLS] bf16, gold [BS, GOLD_W] f32."""
    # step index per (segment, superstep); seg 0 idles (clamped, masked later)
    steps = np.empty((S, TSS), np.int64)
    for s in range(1, S):
        steps[s] = np.arange(TSS) + (s * SEG_LEN - BURN)
    steps[0, :SEG_LEN] = np.arange(SEG_LEN)
    steps[0, SEG_LEN:] = 0  # placeholder, overwritten below

    # F[b, s, t, i] = feats_c[b, steps[s,t], i]
    F = feats_c[:, steps, :]                       # [BS, S, TSS, T]
    F = F - MU
    F[:, 0, SEG_LEN:, :] = 0.0                     # seg-0 idle: fx = 1
    F[:, S - 1, TSS - 1, :] += trans[STOP][None]   # fold STOP transition
    # [BS, S, TSS, T] -> rows (k,i), cols (g, ql, b): s = (g*4+ql)*4 + k
    F = F.reshape(BS, NG, 4, 4, TSS, T)            # [b, g, ql, k, t, i]
    F = F.transpose(3, 5, 4, 1, 2, 0)              # [k, i, t, g, ql, b]
    F = np.ascontiguousarray(F).reshape(128, TSS * COLS)
    staged = F.astype(ml_dtypes.bfloat16)

    gold = np.zeros((BS, GOLD_W), np.float32)
    l_idx = np.arange(L)[None, :]
    b_idx = np.arange(BS)[:, None]
    prev = np.concatenate(
        [np.full((BS, 1), START, tags_c.dtype), tags_c[:, :-1]], axis=1
    )
    gold[:, :L] = feats_c[b_idx, l_idx, tags_c] + trans[tags_c, prev]
    gold[:, L] = trans[STOP, tags_c[:, -1]]
    return staged, gold


LAST_RESULTS = None


def kernel(feats, transitions, tags, _trace=False):
    global _compiled, LAST_RESULTS
    from concourse.bass_utils import run_bass_kernel_spmd

    feats = np.asarray(feats, dtype=np.float32)
    transitions = np.asarray(transitions, dtype=np.float32)
    tags = np.asarray(tags)

    if _compiled is None:
        _compiled = _build_nc()
    nc = _compiled

    idn = np.zeros((4, 10), np.float32)
    idn[np.arange(4), np.arange(4)] = 1.0
    idn[np.arange(4), 4 + np.arange(4)] = -1.0
    idn[:, 8] = 1.0
    idn[:, 9] = -1.0
    in_maps = []
    for c in range(NCORES):
        sl = slice(c * BS, (c + 1) * BS)
        staged, gold = _stage_core(feats[sl], tags[sl], transitions)
        in_maps.append(
            {"staged": staged, "gold": gold, "trans": transitions, "idn": idn}
        )
    res = run_bass_kernel_spmd(
        nc, in_maps, core_ids=list(range(NCORES)), trace=_trace
    )
    LAST_RESULTS = res
    out = np.concatenate([r["out"].reshape(BS) for r in res.results])
    return out.astype(np.float32)
